# revision 42
# baseline (speedup 1.0000x reference)
"""Multi-head self-attention (B=1, S=4096, D=2048, H=16, Dh=128) on 8 TRN2
NeuronCores. Head-sharded tensor parallelism: each core computes 2 heads end to
end in transposed layout, writes its partial out-projection [D, S] in bf16; the
host sums the 8 partials in fp32 and transposes back to [S, D].

Dtype strategy: activations/weights stream as bf16 (matmul inputs), all matmul
accumulation is fp32 in PSUM, softmax statistics (sum-exp, reciprocals, rms
rows) are fp32/fp32r.  Attention scores are computed in S^T layout [k, q] so
the softmax reduction is a ones-column matmul (partition sum) and no transposes
are needed anywhere.  All ones-column reductions run in bf16 (full PE rate).

Host-side layouts are packed per-partition-contiguous so DMA lines are >=2KB:
x as [128, pair(4), kt(16), 1024], weights as [128, kt, m] flat.
"""
import sys
import numpy as np

for _p in ("/opt/trn_rl_repo",):
    if _p not in sys.path:
        sys.path.append(_p)

import concourse.bacc as bacc
import concourse.mybir as mybir
import concourse.tile as tile

F32 = mybir.dt.float32
F32R = mybir.dt.float32r
BF16 = mybir.dt.bfloat16
AF = mybir.ActivationFunctionType
MUL = mybir.AluOpType.mult

D = 2048            # d_model
S = 4096            # sequence length
DH = 128            # head dim
HPC = 2             # heads per core
DHC = HPC * DH      # 256 head-dims per core
NC = 8              # cores
EPS = 1e-6
SCALE = 1.0 / np.sqrt(DH)

NCH = S // 512      # 8 seq chunks of 512
KT_D = D // 128     # 16 k-tiles over d_model
KT_S = S // 128     # 32 k-tiles over sequence
XP = 4              # x pairs (1024-wide) per sequence
KTG = 4             # kt groups of 4 per x DMA

TRACE = False       # set by test harness for profiling runs


def build():
    nc = bacc.Bacc("TRN2", target_bir_lowering=False, debug=False)

    xTb = nc.dram_tensor("xTb", [128, XP * KT_D * 1024], BF16,
                         kind="ExternalInput")
    wqb = nc.dram_tensor("wqb", [128, KT_D * DHC], BF16, kind="ExternalInput")
    wkb = nc.dram_tensor("wkb", [128, KT_D * DHC], BF16, kind="ExternalInput")
    wvb = nc.dram_tensor("wvb", [128, KT_D * DHC], BF16, kind="ExternalInput")
    wob = nc.dram_tensor("wob", [128, HPC * D], BF16, kind="ExternalInput")
    qw = nc.dram_tensor("qw", [DH, 1], F32, kind="ExternalInput")
    kw = nc.dram_tensor("kw", [DH, 1], F32, kind="ExternalInput")
    ones_c_d = nc.dram_tensor("ones_c", [128, 1], BF16, kind="ExternalInput")
    ones_r_d = nc.dram_tensor("ones_r", [1, 128], BF16, kind="ExternalInput")
    outT = nc.dram_tensor("outT", [D, S], BF16, kind="ExternalOutput")

    outT_t = outT.rearrange("(mo p) s -> mo p s", p=128)    # [16,128,4096]

    with tile.TileContext(nc) as tc, \
         nc.allow_low_precision(reason="bf16 compute is intentional"):
        with (
            tc.tile_pool(name="consts", bufs=1) as consts,
            tc.tile_pool(name="big", bufs=1) as big,
            tc.tile_pool(name="stream", bufs=6) as stream,
            tc.tile_pool(name="ev", bufs=1) as ev,
        ):
            # ---- residents ----
            ones_b = consts.tile([128, 1], BF16)           # lhsT for partition-sum
            nc.sync.dma_start(out=ones_b[:], in_=ones_c_d[:])
            ones_r = consts.tile([1, 128], BF16)           # lhsT for PE broadcast
            nc.sync.dma_start(out=ones_r[:], in_=ones_r_d[:])
            eps_sb = consts.tile([1, 1], F32, tag="eps")
            nc.vector.memset(eps_sb[:], EPS)
            qw_sb = consts.tile([DH, 1], F32, tag="qw")    # per-partition norm w
            kw_sb = consts.tile([DH, 1], F32, tag="kw")
            nc.sync.dma_start(out=qw_sb[:], in_=qw[:])
            nc.sync.dma_start(out=kw_sb[:], in_=kw[:])

            qT = [[big.tile([128, 512], BF16, tag=f"q{h}c{n}", name=f"qT{h}_{n}")
                   for n in range(NCH)] for h in range(HPC)]
            kT = [[big.tile([128, 512], BF16, tag=f"k{h}c{n}", name=f"kT{h}_{n}")
                   for n in range(NCH)] for h in range(HPC)]
            v_sb = big.tile([128, KT_S, DHC], BF16, tag="v")
            o_sb = [big.tile([128, S], BF16, tag=f"o{h}", name=f"o{h}")
                    for h in range(HPC)]
            wo_sb = big.tile([128, HPC, D], BF16, tag="wo")

            # ========== Phase 1: q/k/v projections + q/k rmsnorm ==========
            # Single pass over x^T: per (chunk, kt) one x tile feeds 2 q-mms,
            # 2 k-mms and 4 v-mms.  The 4 v accumulators pack two [128,256]
            # groups per PSUM bank.  x streams in [128, ktg(4), 1024] tiles
            # (8KB DMA lines, one pair of 512-chunks per tile).
            with (
                tc.tile_pool(name="wqk", bufs=1) as wqk,
                tc.tile_pool(name="xpool", bufs=1) as xpool,
                tc.tile_pool(name="ps1", bufs=1, space="PSUM") as ps1,
            ):
                # per-group weight tiles so the first matmul's dependency is
                # exactly the first 256KB weight DMA, not the whole tensor
                wq_g = [wqk.tile([128, KTG, DHC], BF16, tag=f"wq{g}",
                                 name=f"wq{g}") for g in range(4)]
                wk_g = [wqk.tile([128, KTG, DHC], BF16, tag=f"wk{g}",
                                 name=f"wk{g}") for g in range(4)]
                wv_g = [wqk.tile([128, KTG, DHC], BF16, tag=f"wv{g}",
                                 name=f"wv{g}") for g in range(4)]

                x_tiles = {}

                def load_x_g(pair, g):
                    t = xpool.tile([128, KTG * 1024], BF16, tag="x",
                                   bufs=5, name=f"x{pair}_{g}")
                    base = pair * KT_D * 1024 + g * KTG * 1024
                    nc.scalar.dma_start(out=t[:],
                                        in_=xTb[:, base:base + KTG * 1024])
                    x_tiles[(pair, g)] = t

                def load_x(pair):
                    for g in range(KTG):
                        load_x_g(pair, g)

                def load_w_g(g):
                    gsl = slice(g * 4 * DHC, (g + 1) * 4 * DHC)
                    nc.sync.dma_start(out=wq_g[g][:].rearrange("p a b -> p (a b)"),
                                      in_=wqb[:, gsl])
                    nc.sync.dma_start(out=wk_g[g][:].rearrange("p a b -> p (a b)"),
                                      in_=wkb[:, gsl])
                    nc.sync.dma_start(out=wv_g[g][:].rearrange("p a b -> p (a b)"),
                                      in_=wvb[:, gsl])

                # DMA issues are staggered just-in-time around chunk 0's
                # matmul groups: the framework's DMA-completion waits are
                # conservative (wait for every DMA issued earlier on the
                # lane), so the first matmul must have few DMAs ahead of it
                load_x_g(0, 0)
                load_w_g(0)

                # HAM warm-up: ~4us of dummy matmuls (inputs are garbage,
                # output discarded) so the PE reaches full clock while the
                # first x/weight DMAs stream
                warm = ps1.tile([1, 512], F32, tag="ss", name="warm", bufs=2)
                for _ in range(24):
                    nc.tensor.matmul(warm[:], ones_b[:], qT[0][0][:],
                                     start=True, stop=True,
                                     skip_group_check=True)

                for n in range(NCH):
                    pair, off = n // 2, (n % 2) * 512
                    if n == 1:
                        nc.sync.dma_start(
                            out=wo_sb[:].rearrange("p a b -> p (a b)"),
                            in_=wob[:])
                    if n % 2 == 0 and pair + 1 < XP and n > 0:
                        load_x(pair + 1)
                    ps_q = [ps1.tile([128, 512], F32, tag=f"psq{m}", name=f"psq{m}")
                            for m in range(HPC)]
                    ps_k = [ps1.tile([128, 512], F32, tag=f"psk{m}", name=f"psk{m}")
                            for m in range(HPC)]
                    ps_v = [ps1.tile([128, 512], F32, tag=f"psv{i}", name=f"psv{i}")
                            for i in range(2)]
                    for g in range(KTG):
                        if n == 0 and g + 1 < KTG:
                            load_x_g(0, g + 1)
                            load_w_g(g + 1)
                        if n == 0 and g == KTG - 1:
                            load_x(1)
                        xt = x_tiles[(pair, g)].rearrange(
                            "p (kt s) -> p kt s", kt=KTG)
                        for kl in range(KTG):
                            kt = g * KTG + kl
                            xs = xt[:, kl, off:off + 512]
                            # q0, q1, k0, k1 first: on the last kt this puts
                            # ~1.3us of k/v matmuls after q0's accumulation
                            # completes, hiding the Square->ones-mm latency
                            for m in range(HPC):
                                ms = slice(m * DH, (m + 1) * DH)
                                nc.tensor.matmul(ps_q[m][:], wq_g[g][:, kl, ms], xs,
                                                 start=(kt == 0), stop=(kt == KT_D - 1),
                                                 skip_group_check=True)
                            for m in range(HPC):
                                ms = slice(m * DH, (m + 1) * DH)
                                nc.tensor.matmul(ps_k[m][:], wk_g[g][:, kl, ms], xs,
                                                 start=(kt == 0), stop=(kt == KT_D - 1),
                                                 skip_group_check=True)
                            for sm in range(4):
                                pv = ps_v[sm // 2][:, (sm % 2) * 256:(sm % 2) * 256 + 256]
                                nc.tensor.matmul(pv,
                                                 xt[:, kl, off + sm * 128:off + (sm + 1) * 128],
                                                 wv_g[g][:, kl, :],
                                                 start=(kt == 0 and sm % 2 == 0),
                                                 stop=(kt == KT_D - 1),
                                                 skip_group_check=True)
                    # rmsnorm + evict q, k; interleaved h0-first so phase 2's
                    # first scores (which need q0/k0) unblock earliest.  The
                    # per-column 1/rms row is broadcast with a PE ones-matmul
                    # into the ss-tag PSUM rotation (tensor is idle here);
                    # no gpsimd in the chain.
                    sl = slice(n * 512, (n + 1) * 512)
                    units = [(ps_q, qT, qw_sb, 0, "s"),
                             (ps_k, kT, kw_sb, 0, "v"),
                             (ps_q, qT, qw_sb, 1, "s"),
                             (ps_k, kT, kw_sb, 1, "v")]
                    if n == NCH - 1:
                        # k0 first: its normalized tile gates phase 2's
                        # first score batch.  All squares on scalar (no
                        # next-chunk matmuls compete here; vector raws run
                        # in parallel)
                        units = [(ps_k, kT, kw_sb, 0, "s"),
                                 (ps_q, qT, qw_sb, 0, "s"),
                                 (ps_k, kT, kw_sb, 1, "s"),
                                 (ps_q, qT, qw_sb, 1, "s")]
                    for ui, (ps_list, dst, w_col, m, sqeng) in enumerate(units):
                        raw = ev.tile([128, 512], F32, tag="raw", bufs=3)
                        nc.vector.tensor_copy(raw[:], ps_list[m][:])
                        sq = ev.tile([128, 512], BF16, tag="sq", bufs=2)
                        if sqeng == "s":
                            nc.scalar.activation(sq[:], ps_list[m][:], AF.Square)
                        else:
                            nc.vector.tensor_mul(sq[:], raw[:], raw[:])
                        ps_ss = ps1.tile([1, 512], F32, tag="ss", name="ps_ss",
                                         bufs=2)
                        nc.tensor.matmul(ps_ss[:], ones_b[:], sq[:],
                                         start=True, stop=True,
                                         skip_group_check=True)
                        ms_row = ev.tile([1, 512], F32, tag="msr", bufs=2)
                        nc.scalar.activation(ms_row[:], ps_ss[:], AF.Identity,
                                             bias=eps_sb[:], scale=1.0 / 128.0)
                        rec = ev.tile([1, 512], F32, tag="rec", bufs=2)
                        nc.vector.reciprocal_approx_fast(out=rec[:], in_=ms_row[:])
                        if n == NCH - 1 and ui == 0:
                            # k0 gates phase 2's first scores: broadcast on
                            # the (idle) PE, skipping gpsimd latency
                            rrms_b = ev.tile([1, 512], BF16, tag="rrms",
                                             bufs=2)
                            nc.scalar.activation(rrms_b[:], rec[:], AF.Sqrt)
                            rb_ps = ps1.tile([128, 512], F32, tag="ss",
                                             name="rb_ps", bufs=2)
                            nc.tensor.matmul(rb_ps[:], ones_r[:], rrms_b[:],
                                             start=True, stop=True,
                                             skip_group_check=True)
                            nc.vector.scalar_tensor_tensor(
                                dst[m][n][:], raw[:], w_col[:], rb_ps[:],
                                op0=MUL, op1=MUL)
                        else:
                            rrms = ev.tile([1, 512], F32R, tag="rrms", bufs=2)
                            nc.scalar.activation(rrms[:], rec[:], AF.Sqrt)
                            rb = ev.tile([128, 512], F32R, tag="rb", bufs=2)
                            nc.gpsimd.partition_broadcast(rb[:], rrms[:])
                            nc.vector.scalar_tensor_tensor(
                                dst[m][n][:], raw[:], w_col[:], rb[:],
                                op0=MUL, op1=MUL)
                    # evict v on vector (scalar stays clear for the squares)
                    for i in range(2):
                        nc.vector.tensor_copy(
                            v_sb[:, n * 4 + 2 * i:n * 4 + 2 * i + 2, :].rearrange(
                                "p a b -> p (a b)"),
                            ps_v[i][:])

            # ============ Phase 2+3: attention + out-projection ============
            # 1024-wide q blocks; S^T scores span two PSUM banks; exp and the
            # sum-exp accumulation amortize per-op overheads over 1024 cols.
            # PSUM budget (per-partition column space, 16KB total):
            #   pss 2x4KB + pso 4KB + y 2x2KB = 16KB.
            with (
                tc.tile_pool(name="ps2", bufs=1, space="PSUM") as ps2,
            ):
                NQB = S // 1024

                def outproj_half(qb, mo, u):
                    # one [128,512] half of the mo-th output row-block
                    mosl = slice(mo * 128, (mo + 1) * 128)
                    usl = slice(qb * 1024 + u * 512, qb * 1024 + (u + 1) * 512)
                    ps_y = ps2.tile([128, 512], F32, tag="y", name="ps_y",
                                    bufs=2)
                    for h2 in range(HPC):
                        nc.tensor.matmul(ps_y[:], wo_sb[:, h2, mosl],
                                         o_sb[h2][:, usl],
                                         start=(h2 == 0), stop=(h2 == HPC - 1),
                                         skip_group_check=True)
                    return ps_y

                def outproj_mo(qb, mo, tail=False):
                    y = stream.tile([128, 1024], BF16, tag="y_ev", bufs=3)
                    for u in range(2):
                        ps_y = outproj_half(qb, mo, u)
                        if tail and u == 0:
                            nc.scalar.copy(y[:, u * 512:(u + 1) * 512], ps_y[:])
                        else:
                            nc.vector.tensor_copy(y[:, u * 512:(u + 1) * 512],
                                                  ps_y[:])
                        if tail:
                            nc.sync.dma_start(
                                out=outT_t[mo][:, qb * 1024 + u * 512:
                                               qb * 1024 + (u + 1) * 512],
                                in_=y[:, u * 512:(u + 1) * 512])
                    if not tail:
                        nc.sync.dma_start(
                            out=outT_t[mo][:, qb * 1024:(qb + 1) * 1024],
                            in_=y[:])

                for qb in range(NQB):
                    qsl = slice(qb * 1024, (qb + 1) * 1024)
                    for h in range(HPC):
                        ps_o = ps2.tile([128, 1024], F32, tag="pso", bufs=1)
                        acc = ev.tile([128, 1024], BF16, tag="acc", bufs=2,
                                      name="acc")
                        pt_prev = None
                        def emit_pv(kt2, pt2):
                            for u in range(2):
                                nc.tensor.matmul(ps_o[:, u * 512:(u + 1) * 512],
                                                 v_sb[:, kt2, h * DH:(h + 1) * DH],
                                                 pt2[:, u * 512:(u + 1) * 512],
                                                 start=(kt2 == 0),
                                                 stop=(kt2 == KT_S - 1),
                                                 skip_group_check=True)

                        pv_pend = []
                        import contextlib
                        for kt in range(KT_S):
                            hp = (tc.high_priority(offset=250)
                                  if (qb == 0 and h == 0 and kt < 4)
                                  else contextlib.nullcontext())
                            hp.__enter__()
                            k_tile = kT[h][kt // 4][:, (kt % 4) * 128:
                                                    (kt % 4 + 1) * 128]
                            ps_s = ps2.tile([128, 1024], F32, tag="pss", bufs=2)
                            for u in range(2):
                                q_tile = qT[h][qb * 2 + u]
                                nc.tensor.matmul(ps_s[:, u * 512:(u + 1) * 512],
                                                 k_tile, q_tile[:],
                                                 start=True, stop=True,
                                                 skip_group_check=True)
                            pt = stream.tile([128, 1024], BF16, tag="pt", bufs=9)
                            nc.scalar.activation(pt[:], ps_s[:], AF.Exp, scale=SCALE)
                            if kt % 2 == 0:
                                pt_prev = pt
                            else:
                                pair = ev.tile([128, 1024], BF16, tag="pair",
                                               bufs=2, name="pair")
                                nc.vector.tensor_add(pair[:], pt_prev[:], pt[:])
                                if kt == 1:
                                    nc.vector.tensor_copy(acc[:], pair[:])
                                else:
                                    nc.vector.tensor_add(acc[:], acc[:], pair[:])
                            pv_pend.append((kt, pt))
                            if len(pv_pend) > 2:
                                emit_pv(*pv_pend.pop(0))
                            hp.__exit__(None, None, None)
                            # interleave previous q-block's out-projection;
                            # deferred to kt>=3 so the qb-boundary softmax
                            # chain doesn't stall the score matmuls
                            if qb > 0:
                                if h == 0 and kt % 2 == 1 and kt >= 3:
                                    outproj_mo(qb - 1, (kt - 3) // 2)
                                elif h == 1 and kt == 1:
                                    outproj_mo(qb - 1, 15)
                        for kt2, pt2 in pv_pend:
                            emit_pv(kt2, pt2)
                        # evict the unnormalized PV sum immediately: frees the
                        # single pso accumulator for the next (qb,h) without
                        # waiting on the softmax-normalize chain
                        final = (qb == NQB - 1 and h == HPC - 1)
                        o_raw = ev.tile([128, 1024], F32, tag="oraw", bufs=1)
                        (nc.scalar.copy if final else nc.vector.tensor_copy)(
                            o_raw[:], ps_o[:])
                        # sum-exp bf16 ones-mm; reciprocal reads the PSUM
                        # row directly (no staging copy)
                        rec2 = ev.tile([1, 1024], F32, tag="rec2", bufs=1)
                        for u in range(2):
                            ps_se = ps2.tile([1, 512], F32, tag="y",
                                             name="ps_se", bufs=2)
                            nc.tensor.matmul(ps_se[:],
                                             ones_b[:],
                                             acc[:, u * 512:(u + 1) * 512],
                                             start=True, stop=True,
                                             skip_group_check=True)
                            nc.vector.reciprocal_approx_fast(
                                out=rec2[:, u * 512:(u + 1) * 512],
                                in_=ps_se[:])
                        if final:
                            # tail out-projection waits on this chain —
                            # broadcast via PE (bf16, 1/4 of the columns)
                            # instead of gpsimd
                            rec2b = ev.tile([1, 1024], BF16, tag="rrms",
                                            bufs=2)
                            nc.vector.tensor_copy(rec2b[:], rec2[:])
                            for u in range(2):
                                rb2_ps = ps2.tile([128, 512], F32, tag="y",
                                                  name="rb2_ps", bufs=2)
                                nc.tensor.matmul(
                                    rb2_ps[:], ones_r[:],
                                    rec2b[:, u * 512:(u + 1) * 512],
                                    start=True, stop=True,
                                    skip_group_check=True)
                                nc.vector.tensor_mul(
                                    o_sb[h][:, qb * 1024 + u * 512:
                                            qb * 1024 + (u + 1) * 512],
                                    o_raw[:, u * 512:(u + 1) * 512],
                                    rb2_ps[:])
                        else:
                            rb2 = ev.tile([128, 1024], F32, tag="rb2", bufs=1)
                            nc.gpsimd.partition_broadcast(rb2[:], rec2[:])
                            nc.vector.tensor_mul(o_sb[h][:, qsl], o_raw[:],
                                                 rb2[:])

                for mo in range(D // 128):
                    outproj_mo(NQB - 1, mo, tail=True)

    nc.compile()
    return nc


_NC_CACHE = None


def _get_nc():
    global _NC_CACHE
    if _NC_CACHE is None:
        _NC_CACHE = build()
    return _NC_CACHE


def _ensure_axon_hooks_stub():
    """bass_utils imports antenv.axon_hooks when tracing is requested via env;
    provide a no-op stub if the image lacks it so a stray BASS_TRACE cannot
    crash the run."""
    import types
    try:
        from antenv import axon_hooks  # noqa: F401
        return
    except Exception:
        pass
    try:
        import antenv
        m = types.ModuleType("antenv.axon_hooks")
        m.set_axon_ntff_profile_hook = lambda h: None
        m.get_axon_ntff_profile_hook = lambda: None
        sys.modules["antenv.axon_hooks"] = m
        antenv.axon_hooks = m
    except Exception:
        pass


def kernel(x, wq, wk, wv, wo, q_norm_w, k_norm_w):
    import ml_dtypes
    from concourse import bass_utils

    _ensure_axon_hooks_stub()

    x = np.asarray(x, dtype=np.float32)
    wq = np.asarray(wq, dtype=np.float32)
    wk = np.asarray(wk, dtype=np.float32)
    wv = np.asarray(wv, dtype=np.float32)
    wo = np.asarray(wo, dtype=np.float32)
    q_norm_w = np.asarray(q_norm_w, dtype=np.float32).reshape(DH, 1)
    k_norm_w = np.asarray(k_norm_w, dtype=np.float32).reshape(DH, 1)

    B = x.shape[0]
    # x^T packed [p, pair, kt, 1024] so DMA lines are 8KB
    xT = np.ascontiguousarray(x.reshape(S, D).T)
    xprep = np.ascontiguousarray(
        xT.reshape(KT_D, 128, XP, 1024).transpose(1, 2, 0, 3).reshape(
            128, XP * KT_D * 1024)).astype(ml_dtypes.bfloat16)

    def prep_w(wc):          # [2048, DHC] -> [128, kt*DHC] kt-packed
        return np.ascontiguousarray(
            wc.reshape(KT_D, 128, DHC).transpose(1, 0, 2).reshape(
                128, KT_D * DHC)).astype(ml_dtypes.bfloat16)

    in_maps = []
    for c in range(NC):
        hsl = slice(c * DHC, (c + 1) * DHC)
        woc = wo[:, hsl].T    # [DHC, D]
        wo_prep = np.ascontiguousarray(
            woc.reshape(HPC, 128, D).transpose(1, 0, 2).reshape(
                128, HPC * D)).astype(ml_dtypes.bfloat16)
        in_maps.append({
            "xTb": xprep,
            "wqb": prep_w(np.ascontiguousarray(wq[hsl, :].T)),
            "wkb": prep_w(np.ascontiguousarray(wk[hsl, :].T)),
            "wvb": prep_w(np.ascontiguousarray(wv[hsl, :].T)),
            "wob": wo_prep,
            "qw": q_norm_w,
            "kw": k_norm_w,
            "ones_c": np.ones((128, 1), dtype=ml_dtypes.bfloat16),
            "ones_r": np.ones((1, 128), dtype=ml_dtypes.bfloat16),
        })

    nc = _get_nc()
    res = bass_utils.run_bass_kernel_spmd(
        nc, in_maps, core_ids=list(range(NC)), trace=TRACE,
    )
    acc = np.zeros((D, S), dtype=np.float32)
    for c in range(NC):
        acc += res.results[c]["outT"].astype(np.float32)
    out = np.ascontiguousarray(acc.T).reshape(B, S, D)
    if TRACE:
        kernel.last_exec_time_ns = res.exec_time_ns
        kernel.last_results = res
    return out


# revision 43
# speedup vs baseline: 1.0004x; 1.0004x over previous
"""Multi-head self-attention (B=1, S=4096, D=2048, H=16, Dh=128) on 8 TRN2
NeuronCores. Head-sharded tensor parallelism: each core computes 2 heads end to
end in transposed layout, writes its partial out-projection [D, S] in bf16; the
host sums the 8 partials in fp32 and transposes back to [S, D].

Dtype strategy: activations/weights stream as bf16 (matmul inputs), all matmul
accumulation is fp32 in PSUM, softmax statistics (sum-exp, reciprocals, rms
rows) are fp32/fp32r.  Attention scores are computed in S^T layout [k, q] so
the softmax reduction is a ones-column matmul (partition sum) and no transposes
are needed anywhere.  All ones-column reductions run in bf16 (full PE rate).

Host-side layouts are packed per-partition-contiguous so DMA lines are >=2KB:
x as [128, pair(4), kt(16), 1024], weights as [128, kt, m] flat.
"""
import sys
import numpy as np

for _p in ("/opt/trn_rl_repo",):
    if _p not in sys.path:
        sys.path.append(_p)

import concourse.bacc as bacc
import concourse.mybir as mybir
import concourse.tile as tile

F32 = mybir.dt.float32
F32R = mybir.dt.float32r
BF16 = mybir.dt.bfloat16
AF = mybir.ActivationFunctionType
MUL = mybir.AluOpType.mult

D = 2048            # d_model
S = 4096            # sequence length
DH = 128            # head dim
HPC = 2             # heads per core
DHC = HPC * DH      # 256 head-dims per core
NC = 8              # cores
EPS = 1e-6
SCALE = 1.0 / np.sqrt(DH)

NCH = S // 512      # 8 seq chunks of 512
KT_D = D // 128     # 16 k-tiles over d_model
KT_S = S // 128     # 32 k-tiles over sequence
XP = 4              # x pairs (1024-wide) per sequence
KTG = 4             # kt groups of 4 per x DMA

TRACE = False       # set by test harness for profiling runs


def build():
    nc = bacc.Bacc("TRN2", target_bir_lowering=False, debug=False)

    xTb = nc.dram_tensor("xTb", [128, XP * KT_D * 1024], BF16,
                         kind="ExternalInput")
    wqb = nc.dram_tensor("wqb", [128, KT_D * DHC], BF16, kind="ExternalInput")
    wkb = nc.dram_tensor("wkb", [128, KT_D * DHC], BF16, kind="ExternalInput")
    wvb = nc.dram_tensor("wvb", [128, KT_D * DHC], BF16, kind="ExternalInput")
    wob = nc.dram_tensor("wob", [128, HPC * D], BF16, kind="ExternalInput")
    qw = nc.dram_tensor("qw", [DH, 1], F32, kind="ExternalInput")
    kw = nc.dram_tensor("kw", [DH, 1], F32, kind="ExternalInput")
    ones_c_d = nc.dram_tensor("ones_c", [128, 1], BF16, kind="ExternalInput")
    ones_r_d = nc.dram_tensor("ones_r", [1, 128], BF16, kind="ExternalInput")
    outT = nc.dram_tensor("outT", [D, S], BF16, kind="ExternalOutput")

    outT_t = outT.rearrange("(mo p) s -> mo p s", p=128)    # [16,128,4096]

    with tile.TileContext(nc) as tc, \
         nc.allow_low_precision(reason="bf16 compute is intentional"):
        with (
            tc.tile_pool(name="consts", bufs=1) as consts,
            tc.tile_pool(name="big", bufs=1) as big,
            tc.tile_pool(name="stream", bufs=6) as stream,
            tc.tile_pool(name="ev", bufs=1) as ev,
        ):
            # ---- residents ----
            ones_b = consts.tile([128, 1], BF16)           # lhsT for partition-sum
            nc.sync.dma_start(out=ones_b[:], in_=ones_c_d[:])
            ones_r = consts.tile([1, 128], BF16)           # lhsT for PE broadcast
            nc.sync.dma_start(out=ones_r[:], in_=ones_r_d[:])
            eps_sb = consts.tile([1, 1], F32, tag="eps")
            nc.vector.memset(eps_sb[:], EPS)
            qw_sb = consts.tile([DH, 1], F32, tag="qw")    # per-partition norm w
            kw_sb = consts.tile([DH, 1], F32, tag="kw")
            nc.sync.dma_start(out=qw_sb[:], in_=qw[:])
            nc.sync.dma_start(out=kw_sb[:], in_=kw[:])

            qT = [[big.tile([128, 512], BF16, tag=f"q{h}c{n}", name=f"qT{h}_{n}")
                   for n in range(NCH)] for h in range(HPC)]
            kT = [[big.tile([128, 512], BF16, tag=f"k{h}c{n}", name=f"kT{h}_{n}")
                   for n in range(NCH)] for h in range(HPC)]
            v_sb = big.tile([128, KT_S, DHC], BF16, tag="v")
            o_sb = [big.tile([128, S], BF16, tag=f"o{h}", name=f"o{h}")
                    for h in range(HPC)]
            wo_sb = big.tile([128, HPC, D], BF16, tag="wo")

            # ========== Phase 1: q/k/v projections + q/k rmsnorm ==========
            # Single pass over x^T: per (chunk, kt) one x tile feeds 2 q-mms,
            # 2 k-mms and 4 v-mms.  The 4 v accumulators pack two [128,256]
            # groups per PSUM bank.  x streams in [128, ktg(4), 1024] tiles
            # (8KB DMA lines, one pair of 512-chunks per tile).
            with (
                tc.tile_pool(name="wqk", bufs=1) as wqk,
                tc.tile_pool(name="xpool", bufs=1) as xpool,
                tc.tile_pool(name="ps1", bufs=1, space="PSUM") as ps1,
            ):
                # per-group weight tiles so the first matmul's dependency is
                # exactly the first 256KB weight DMA, not the whole tensor
                wq_g = [wqk.tile([128, KTG, DHC], BF16, tag=f"wq{g}",
                                 name=f"wq{g}") for g in range(4)]
                wk_g = [wqk.tile([128, KTG, DHC], BF16, tag=f"wk{g}",
                                 name=f"wk{g}") for g in range(4)]
                wv_g = [wqk.tile([128, KTG, DHC], BF16, tag=f"wv{g}",
                                 name=f"wv{g}") for g in range(4)]

                x_tiles = {}

                def load_x_g(pair, g):
                    t = xpool.tile([128, KTG * 1024], BF16, tag="x",
                                   bufs=5, name=f"x{pair}_{g}")
                    base = pair * KT_D * 1024 + g * KTG * 1024
                    nc.scalar.dma_start(out=t[:],
                                        in_=xTb[:, base:base + KTG * 1024])
                    x_tiles[(pair, g)] = t

                def load_x(pair):
                    for g in range(KTG):
                        load_x_g(pair, g)

                def load_w_g(g):
                    gsl = slice(g * 4 * DHC, (g + 1) * 4 * DHC)
                    nc.sync.dma_start(out=wq_g[g][:].rearrange("p a b -> p (a b)"),
                                      in_=wqb[:, gsl])
                    nc.sync.dma_start(out=wk_g[g][:].rearrange("p a b -> p (a b)"),
                                      in_=wkb[:, gsl])
                    nc.sync.dma_start(out=wv_g[g][:].rearrange("p a b -> p (a b)"),
                                      in_=wvb[:, gsl])

                # DMA issues are staggered just-in-time around chunk 0's
                # matmul groups: the framework's DMA-completion waits are
                # conservative (wait for every DMA issued earlier on the
                # lane), so the first matmul must have few DMAs ahead of it
                load_x_g(0, 0)
                load_w_g(0)

                # HAM warm-up: ~4us of dummy matmuls (inputs are garbage,
                # output discarded) so the PE reaches full clock while the
                # first x/weight DMAs stream
                warm = ps1.tile([1, 512], F32, tag="ss", name="warm", bufs=2)
                for _ in range(24):
                    nc.tensor.matmul(warm[:], ones_b[:], qT[0][0][:],
                                     start=True, stop=True,
                                     skip_group_check=True)

                for n in range(NCH):
                    pair, off = n // 2, (n % 2) * 512
                    if n == 1:
                        nc.sync.dma_start(
                            out=wo_sb[:].rearrange("p a b -> p (a b)"),
                            in_=wob[:])
                    if n % 2 == 0 and pair + 1 < XP and n > 0:
                        load_x(pair + 1)
                    ps_q = [ps1.tile([128, 512], F32, tag=f"psq{m}", name=f"psq{m}")
                            for m in range(HPC)]
                    ps_k = [ps1.tile([128, 512], F32, tag=f"psk{m}", name=f"psk{m}")
                            for m in range(HPC)]
                    ps_v = [ps1.tile([128, 512], F32, tag=f"psv{i}", name=f"psv{i}")
                            for i in range(2)]
                    for g in range(KTG):
                        if n == 0 and g + 1 < KTG:
                            load_x_g(0, g + 1)
                            load_w_g(g + 1)
                        if n == 0 and g == KTG - 1:
                            load_x(1)
                        xt = x_tiles[(pair, g)].rearrange(
                            "p (kt s) -> p kt s", kt=KTG)
                        for kl in range(KTG):
                            kt = g * KTG + kl
                            xs = xt[:, kl, off:off + 512]
                            # q0, q1, k0, k1 first: on the last kt this puts
                            # ~1.3us of k/v matmuls after q0's accumulation
                            # completes, hiding the Square->ones-mm latency
                            for m in range(HPC):
                                ms = slice(m * DH, (m + 1) * DH)
                                nc.tensor.matmul(ps_q[m][:], wq_g[g][:, kl, ms], xs,
                                                 start=(kt == 0), stop=(kt == KT_D - 1),
                                                 skip_group_check=True)
                            for m in range(HPC):
                                ms = slice(m * DH, (m + 1) * DH)
                                nc.tensor.matmul(ps_k[m][:], wk_g[g][:, kl, ms], xs,
                                                 start=(kt == 0), stop=(kt == KT_D - 1),
                                                 skip_group_check=True)
                            for sm in range(4):
                                pv = ps_v[sm // 2][:, (sm % 2) * 256:(sm % 2) * 256 + 256]
                                nc.tensor.matmul(pv,
                                                 xt[:, kl, off + sm * 128:off + (sm + 1) * 128],
                                                 wv_g[g][:, kl, :],
                                                 start=(kt == 0 and sm % 2 == 0),
                                                 stop=(kt == KT_D - 1),
                                                 skip_group_check=True)
                    # rmsnorm + evict q, k; interleaved h0-first so phase 2's
                    # first scores (which need q0/k0) unblock earliest.  The
                    # per-column 1/rms row is broadcast with a PE ones-matmul
                    # into the ss-tag PSUM rotation (tensor is idle here);
                    # no gpsimd in the chain.
                    sl = slice(n * 512, (n + 1) * 512)
                    units = [(ps_q, qT, qw_sb, 0, "s"),
                             (ps_k, kT, kw_sb, 0, "v"),
                             (ps_q, qT, qw_sb, 1, "s"),
                             (ps_k, kT, kw_sb, 1, "v")]
                    if n == NCH - 1:
                        # k0 first: its normalized tile gates phase 2's
                        # first score batch.  All squares on scalar (no
                        # next-chunk matmuls compete here; vector raws run
                        # in parallel)
                        units = [(ps_k, kT, kw_sb, 0, "s"),
                                 (ps_q, qT, qw_sb, 0, "s"),
                                 (ps_k, kT, kw_sb, 1, "s"),
                                 (ps_q, qT, qw_sb, 1, "s")]
                    for ui, (ps_list, dst, w_col, m, sqeng) in enumerate(units):
                        raw = ev.tile([128, 512], F32, tag="raw", bufs=3)
                        nc.vector.tensor_copy(raw[:], ps_list[m][:])
                        sq = ev.tile([128, 512], BF16, tag="sq", bufs=2)
                        if sqeng == "s":
                            nc.scalar.activation(sq[:], ps_list[m][:], AF.Square)
                        else:
                            nc.vector.tensor_mul(sq[:], raw[:], raw[:])
                        ps_ss = ps1.tile([1, 512], F32, tag="ss", name="ps_ss",
                                         bufs=2)
                        nc.tensor.matmul(ps_ss[:], ones_b[:], sq[:],
                                         start=True, stop=True,
                                         skip_group_check=True)
                        ms_row = ev.tile([1, 512], F32, tag="msr", bufs=2)
                        nc.scalar.activation(ms_row[:], ps_ss[:], AF.Identity,
                                             bias=eps_sb[:], scale=1.0 / 128.0)
                        rec = ev.tile([1, 512], F32, tag="rec", bufs=2)
                        nc.vector.reciprocal_approx_fast(out=rec[:], in_=ms_row[:])
                        if n == NCH - 1 and ui == 0:
                            # k0 gates phase 2's first scores: broadcast on
                            # the (idle) PE, skipping gpsimd latency
                            rrms_b = ev.tile([1, 512], BF16, tag="rrms",
                                             bufs=2)
                            nc.scalar.activation(rrms_b[:], rec[:], AF.Sqrt)
                            rb_ps = ps1.tile([128, 512], F32, tag="ss",
                                             name="rb_ps", bufs=2)
                            nc.tensor.matmul(rb_ps[:], ones_r[:], rrms_b[:],
                                             start=True, stop=True,
                                             skip_group_check=True)
                            nc.vector.scalar_tensor_tensor(
                                dst[m][n][:], raw[:], w_col[:], rb_ps[:],
                                op0=MUL, op1=MUL)
                        else:
                            rrms = ev.tile([1, 512], F32R, tag="rrms", bufs=2)
                            nc.scalar.activation(rrms[:], rec[:], AF.Sqrt)
                            rb = ev.tile([128, 512], F32R, tag="rb", bufs=2)
                            nc.gpsimd.partition_broadcast(rb[:], rrms[:])
                            nc.vector.scalar_tensor_tensor(
                                dst[m][n][:], raw[:], w_col[:], rb[:],
                                op0=MUL, op1=MUL)
                    # evict v on vector (scalar stays clear for the squares)
                    for i in range(2):
                        nc.vector.tensor_copy(
                            v_sb[:, n * 4 + 2 * i:n * 4 + 2 * i + 2, :].rearrange(
                                "p a b -> p (a b)"),
                            ps_v[i][:])

            # ============ Phase 2+3: attention + out-projection ============
            # 1024-wide q blocks; S^T scores span two PSUM banks; exp and the
            # sum-exp accumulation amortize per-op overheads over 1024 cols.
            # PSUM budget (per-partition column space, 16KB total):
            #   pss 2x4KB + pso 4KB + y 2x2KB = 16KB.
            with (
                tc.tile_pool(name="ps2", bufs=1, space="PSUM") as ps2,
            ):
                NQB = S // 1024

                def outproj_half(qb, mo, u):
                    # one [128,512] half of the mo-th output row-block
                    mosl = slice(mo * 128, (mo + 1) * 128)
                    usl = slice(qb * 1024 + u * 512, qb * 1024 + (u + 1) * 512)
                    ps_y = ps2.tile([128, 512], F32, tag="y", name="ps_y",
                                    bufs=2)
                    for h2 in range(HPC):
                        nc.tensor.matmul(ps_y[:], wo_sb[:, h2, mosl],
                                         o_sb[h2][:, usl],
                                         start=(h2 == 0), stop=(h2 == HPC - 1),
                                         skip_group_check=True)
                    return ps_y

                def outproj_mo(qb, mo, tail=False):
                    y = stream.tile([128, 1024], BF16, tag="y_ev", bufs=3)
                    for u in range(2):
                        ps_y = outproj_half(qb, mo, u)
                        if tail and u == 0:
                            nc.scalar.copy(y[:, u * 512:(u + 1) * 512], ps_y[:])
                        else:
                            nc.vector.tensor_copy(y[:, u * 512:(u + 1) * 512],
                                                  ps_y[:])
                        if tail:
                            nc.sync.dma_start(
                                out=outT_t[mo][:, qb * 1024 + u * 512:
                                               qb * 1024 + (u + 1) * 512],
                                in_=y[:, u * 512:(u + 1) * 512])
                    if not tail:
                        nc.sync.dma_start(
                            out=outT_t[mo][:, qb * 1024:(qb + 1) * 1024],
                            in_=y[:])

                for qb in range(NQB):
                    qsl = slice(qb * 1024, (qb + 1) * 1024)
                    for h in range(HPC):
                        ps_o = ps2.tile([128, 1024], F32, tag="pso", bufs=1)
                        acc = ev.tile([128, 1024], BF16, tag="acc", bufs=2,
                                      name="acc")
                        pt_prev = None
                        def emit_pv(kt2, pt2):
                            for u in range(2):
                                nc.tensor.matmul(ps_o[:, u * 512:(u + 1) * 512],
                                                 v_sb[:, kt2, h * DH:(h + 1) * DH],
                                                 pt2[:, u * 512:(u + 1) * 512],
                                                 start=(kt2 == 0),
                                                 stop=(kt2 == KT_S - 1),
                                                 skip_group_check=True)

                        pv_pend = []
                        for kt in range(KT_S):
                            k_tile = kT[h][kt // 4][:, (kt % 4) * 128:
                                                    (kt % 4 + 1) * 128]
                            ps_s = ps2.tile([128, 1024], F32, tag="pss", bufs=2)
                            for u in range(2):
                                q_tile = qT[h][qb * 2 + u]
                                nc.tensor.matmul(ps_s[:, u * 512:(u + 1) * 512],
                                                 k_tile, q_tile[:],
                                                 start=True, stop=True,
                                                 skip_group_check=True)
                            pt = stream.tile([128, 1024], BF16, tag="pt", bufs=9)
                            nc.scalar.activation(pt[:], ps_s[:], AF.Exp, scale=SCALE)
                            if kt % 2 == 0:
                                pt_prev = pt
                            else:
                                pair = ev.tile([128, 1024], BF16, tag="pair",
                                               bufs=2, name="pair")
                                nc.vector.tensor_add(pair[:], pt_prev[:], pt[:])
                                if kt == 1:
                                    nc.vector.tensor_copy(acc[:], pair[:])
                                else:
                                    nc.vector.tensor_add(acc[:], acc[:], pair[:])
                            pv_pend.append((kt, pt))
                            if len(pv_pend) > 2:
                                emit_pv(*pv_pend.pop(0))
                            # interleave previous q-block's out-projection;
                            # deferred to kt>=3 so the qb-boundary softmax
                            # chain doesn't stall the score matmuls
                            if qb > 0:
                                if h == 0 and kt % 2 == 1 and kt >= 3:
                                    outproj_mo(qb - 1, (kt - 3) // 2)
                                elif h == 1 and kt == 1:
                                    outproj_mo(qb - 1, 15)
                        for kt2, pt2 in pv_pend:
                            emit_pv(kt2, pt2)
                        # evict the unnormalized PV sum immediately: frees the
                        # single pso accumulator for the next (qb,h) without
                        # waiting on the softmax-normalize chain
                        final = (qb == NQB - 1 and h == HPC - 1)
                        o_raw = ev.tile([128, 1024], F32, tag="oraw", bufs=1)
                        (nc.scalar.copy if final else nc.vector.tensor_copy)(
                            o_raw[:], ps_o[:])
                        # sum-exp bf16 ones-mm; reciprocal reads the PSUM
                        # row directly (no staging copy)
                        rec2 = ev.tile([1, 1024], F32, tag="rec2", bufs=1)
                        for u in range(2):
                            ps_se = ps2.tile([1, 512], F32, tag="y",
                                             name="ps_se", bufs=2)
                            nc.tensor.matmul(ps_se[:],
                                             ones_b[:],
                                             acc[:, u * 512:(u + 1) * 512],
                                             start=True, stop=True,
                                             skip_group_check=True)
                            nc.vector.reciprocal_approx_fast(
                                out=rec2[:, u * 512:(u + 1) * 512],
                                in_=ps_se[:])
                        if final:
                            # tail out-projection waits on this chain —
                            # broadcast via PE (bf16, 1/4 of the columns)
                            # instead of gpsimd
                            rec2b = ev.tile([1, 1024], BF16, tag="rrms",
                                            bufs=2)
                            nc.vector.tensor_copy(rec2b[:], rec2[:])
                            for u in range(2):
                                rb2_ps = ps2.tile([128, 512], F32, tag="y",
                                                  name="rb2_ps", bufs=2)
                                nc.tensor.matmul(
                                    rb2_ps[:], ones_r[:],
                                    rec2b[:, u * 512:(u + 1) * 512],
                                    start=True, stop=True,
                                    skip_group_check=True)
                                nc.vector.tensor_mul(
                                    o_sb[h][:, qb * 1024 + u * 512:
                                            qb * 1024 + (u + 1) * 512],
                                    o_raw[:, u * 512:(u + 1) * 512],
                                    rb2_ps[:])
                        else:
                            rb2 = ev.tile([128, 1024], F32, tag="rb2", bufs=1)
                            nc.gpsimd.partition_broadcast(rb2[:], rec2[:])
                            nc.vector.tensor_mul(o_sb[h][:, qsl], o_raw[:],
                                                 rb2[:])

                for mo in range(D // 128):
                    outproj_mo(NQB - 1, mo, tail=True)

    nc.compile()
    return nc


_NC_CACHE = None


def _get_nc():
    global _NC_CACHE
    if _NC_CACHE is None:
        _NC_CACHE = build()
    return _NC_CACHE


def _ensure_axon_hooks_stub():
    """bass_utils imports antenv.axon_hooks when tracing is requested via env;
    provide a no-op stub if the image lacks it so a stray BASS_TRACE cannot
    crash the run."""
    import types
    try:
        from antenv import axon_hooks  # noqa: F401
        return
    except Exception:
        pass
    try:
        import antenv
        m = types.ModuleType("antenv.axon_hooks")
        m.set_axon_ntff_profile_hook = lambda h: None
        m.get_axon_ntff_profile_hook = lambda: None
        sys.modules["antenv.axon_hooks"] = m
        antenv.axon_hooks = m
    except Exception:
        pass


def kernel(x, wq, wk, wv, wo, q_norm_w, k_norm_w):
    import ml_dtypes
    from concourse import bass_utils

    _ensure_axon_hooks_stub()

    x = np.asarray(x, dtype=np.float32)
    wq = np.asarray(wq, dtype=np.float32)
    wk = np.asarray(wk, dtype=np.float32)
    wv = np.asarray(wv, dtype=np.float32)
    wo = np.asarray(wo, dtype=np.float32)
    q_norm_w = np.asarray(q_norm_w, dtype=np.float32).reshape(DH, 1)
    k_norm_w = np.asarray(k_norm_w, dtype=np.float32).reshape(DH, 1)

    B = x.shape[0]
    # x^T packed [p, pair, kt, 1024] so DMA lines are 8KB
    xT = np.ascontiguousarray(x.reshape(S, D).T)
    xprep = np.ascontiguousarray(
        xT.reshape(KT_D, 128, XP, 1024).transpose(1, 2, 0, 3).reshape(
            128, XP * KT_D * 1024)).astype(ml_dtypes.bfloat16)

    def prep_w(wc):          # [2048, DHC] -> [128, kt*DHC] kt-packed
        return np.ascontiguousarray(
            wc.reshape(KT_D, 128, DHC).transpose(1, 0, 2).reshape(
                128, KT_D * DHC)).astype(ml_dtypes.bfloat16)

    in_maps = []
    for c in range(NC):
        hsl = slice(c * DHC, (c + 1) * DHC)
        woc = wo[:, hsl].T    # [DHC, D]
        wo_prep = np.ascontiguousarray(
            woc.reshape(HPC, 128, D).transpose(1, 0, 2).reshape(
                128, HPC * D)).astype(ml_dtypes.bfloat16)
        in_maps.append({
            "xTb": xprep,
            "wqb": prep_w(np.ascontiguousarray(wq[hsl, :].T)),
            "wkb": prep_w(np.ascontiguousarray(wk[hsl, :].T)),
            "wvb": prep_w(np.ascontiguousarray(wv[hsl, :].T)),
            "wob": wo_prep,
            "qw": q_norm_w,
            "kw": k_norm_w,
            "ones_c": np.ones((128, 1), dtype=ml_dtypes.bfloat16),
            "ones_r": np.ones((1, 128), dtype=ml_dtypes.bfloat16),
        })

    nc = _get_nc()
    res = bass_utils.run_bass_kernel_spmd(
        nc, in_maps, core_ids=list(range(NC)), trace=TRACE,
    )
    acc = np.zeros((D, S), dtype=np.float32)
    for c in range(NC):
        acc += res.results[c]["outT"].astype(np.float32)
    out = np.ascontiguousarray(acc.T).reshape(B, S, D)
    if TRACE:
        kernel.last_exec_time_ns = res.exec_time_ns
        kernel.last_results = res
    return out


# revision 44
# speedup vs baseline: 1.0165x; 1.0161x over previous
"""Multi-head self-attention (B=1, S=4096, D=2048, H=16, Dh=128) on 8 TRN2
NeuronCores. Head-sharded tensor parallelism: each core computes 2 heads end to
end in transposed layout, writes its partial out-projection [D, S] in bf16; the
host sums the 8 partials in fp32 and transposes back to [S, D].

Dtype strategy: activations/weights stream as bf16 (matmul inputs), all matmul
accumulation is fp32 in PSUM, softmax statistics (sum-exp, reciprocals, rms
rows) are fp32/fp32r.  Attention scores are computed in S^T layout [k, q] so
the softmax reduction is a ones-column matmul (partition sum) and no transposes
are needed anywhere.  All ones-column reductions run in bf16 (full PE rate).

Host-side layouts are packed per-partition-contiguous so DMA lines are >=2KB:
x as [128, pair(4), kt(16), 1024], weights as [128, kt, m] flat.
"""
import sys
import numpy as np

for _p in ("/opt/trn_rl_repo",):
    if _p not in sys.path:
        sys.path.append(_p)

import concourse.bacc as bacc
import concourse.mybir as mybir
import concourse.tile as tile

F32 = mybir.dt.float32
F32R = mybir.dt.float32r
BF16 = mybir.dt.bfloat16
AF = mybir.ActivationFunctionType
MUL = mybir.AluOpType.mult

D = 2048            # d_model
S = 4096            # sequence length
DH = 128            # head dim
HPC = 2             # heads per core
DHC = HPC * DH      # 256 head-dims per core
NC = 8              # cores
EPS = 1e-6
SCALE = 1.0 / np.sqrt(DH)

NCH = S // 512      # 8 seq chunks of 512
KT_D = D // 128     # 16 k-tiles over d_model
KT_S = S // 128     # 32 k-tiles over sequence
XP = 4              # x pairs (1024-wide) per sequence
KTG = 4             # kt groups of 4 per x DMA

TRACE = False       # set by test harness for profiling runs


def build():
    nc = bacc.Bacc("TRN2", target_bir_lowering=False, debug=False)

    xTb = nc.dram_tensor("xTb", [128, XP * KT_D * 1024], BF16,
                         kind="ExternalInput")
    wqb = nc.dram_tensor("wqb", [128, KT_D * DHC], BF16, kind="ExternalInput")
    wkb = nc.dram_tensor("wkb", [128, KT_D * DHC], BF16, kind="ExternalInput")
    wvb = nc.dram_tensor("wvb", [128, KT_D * DHC], BF16, kind="ExternalInput")
    wob = nc.dram_tensor("wob", [128, HPC * D], BF16, kind="ExternalInput")
    qw = nc.dram_tensor("qw", [DH, 1], F32, kind="ExternalInput")
    kw = nc.dram_tensor("kw", [DH, 1], F32, kind="ExternalInput")
    ones_c_d = nc.dram_tensor("ones_c", [128, 1], BF16, kind="ExternalInput")
    ones_r_d = nc.dram_tensor("ones_r", [1, 128], BF16, kind="ExternalInput")
    outT = nc.dram_tensor("outT", [D, S], BF16, kind="ExternalOutput")

    outT_t = outT.rearrange("(mo p) s -> mo p s", p=128)    # [16,128,4096]

    with tile.TileContext(nc) as tc, \
         nc.allow_low_precision(reason="bf16 compute is intentional"):
        with (
            tc.tile_pool(name="consts", bufs=1) as consts,
            tc.tile_pool(name="big", bufs=1) as big,
            tc.tile_pool(name="stream", bufs=6) as stream,
            tc.tile_pool(name="ev", bufs=1) as ev,
        ):
            # ---- residents ----
            ones_b = consts.tile([128, 1], BF16)           # lhsT for partition-sum
            nc.sync.dma_start(out=ones_b[:], in_=ones_c_d[:])
            ones_r = consts.tile([1, 128], BF16)           # lhsT for PE broadcast
            nc.sync.dma_start(out=ones_r[:], in_=ones_r_d[:])
            eps_sb = consts.tile([1, 1], F32, tag="eps")
            nc.vector.memset(eps_sb[:], EPS)
            qw_sb = consts.tile([DH, 1], F32, tag="qw")    # per-partition norm w
            kw_sb = consts.tile([DH, 1], F32, tag="kw")
            nc.sync.dma_start(out=qw_sb[:], in_=qw[:])
            nc.sync.dma_start(out=kw_sb[:], in_=kw[:])

            qT = [[big.tile([128, 512], BF16, tag=f"q{h}c{n}", name=f"qT{h}_{n}")
                   for n in range(NCH)] for h in range(HPC)]
            kT = [[big.tile([128, 512], BF16, tag=f"k{h}c{n}", name=f"kT{h}_{n}")
                   for n in range(NCH)] for h in range(HPC)]
            v_sb = big.tile([128, KT_S, DHC], BF16, tag="v")
            o_sb = [big.tile([128, S], BF16, tag=f"o{h}", name=f"o{h}")
                    for h in range(HPC)]
            wo_sb = big.tile([128, HPC, D], BF16, tag="wo")

            # ========== Phase 1: q/k/v projections + q/k rmsnorm ==========
            # Single pass over x^T: per (chunk, kt) one x tile feeds 2 q-mms,
            # 2 k-mms and 4 v-mms.  The 4 v accumulators pack two [128,256]
            # groups per PSUM bank.  x streams in [128, ktg(4), 1024] tiles
            # (8KB DMA lines, one pair of 512-chunks per tile).
            with (
                tc.tile_pool(name="wqk", bufs=1) as wqk,
                tc.tile_pool(name="xpool", bufs=1) as xpool,
                tc.tile_pool(name="ps1", bufs=1, space="PSUM") as ps1,
            ):
                # per-group weight tiles so the first matmul's dependency is
                # exactly the first 256KB weight DMA, not the whole tensor
                wq_g = [wqk.tile([128, KTG, DHC], BF16, tag=f"wq{g}",
                                 name=f"wq{g}") for g in range(4)]
                wk_g = [wqk.tile([128, KTG, DHC], BF16, tag=f"wk{g}",
                                 name=f"wk{g}") for g in range(4)]
                wv_g = [wqk.tile([128, KTG, DHC], BF16, tag=f"wv{g}",
                                 name=f"wv{g}") for g in range(4)]

                x_tiles = {}

                def load_x_g(pair, g):
                    t = xpool.tile([128, KTG * 1024], BF16, tag="x",
                                   bufs=5, name=f"x{pair}_{g}")
                    base = pair * KT_D * 1024 + g * KTG * 1024
                    nc.scalar.dma_start(out=t[:],
                                        in_=xTb[:, base:base + KTG * 1024])
                    x_tiles[(pair, g)] = t

                def load_x(pair):
                    for g in range(KTG):
                        load_x_g(pair, g)

                def load_w_g(g):
                    gsl = slice(g * 4 * DHC, (g + 1) * 4 * DHC)
                    nc.sync.dma_start(out=wq_g[g][:].rearrange("p a b -> p (a b)"),
                                      in_=wqb[:, gsl])
                    nc.sync.dma_start(out=wk_g[g][:].rearrange("p a b -> p (a b)"),
                                      in_=wkb[:, gsl])
                    nc.sync.dma_start(out=wv_g[g][:].rearrange("p a b -> p (a b)"),
                                      in_=wvb[:, gsl])

                # DMA issues are staggered just-in-time around chunk 0's
                # matmul groups: the framework's DMA-completion waits are
                # conservative (wait for every DMA issued earlier on the
                # lane), so the first matmul must have few DMAs ahead of it
                load_x_g(0, 0)
                load_w_g(0)

                # HAM warm-up: ~4us of dummy matmuls (inputs are garbage,
                # output discarded) so the PE reaches full clock while the
                # first x/weight DMAs stream
                warm = ps1.tile([1, 512], F32, tag="ss", name="warm", bufs=2)
                for _ in range(24):
                    nc.tensor.matmul(warm[:], ones_b[:], qT[0][0][:],
                                     start=True, stop=True,
                                     skip_group_check=True)

                for n in range(NCH):
                    pair, off = n // 2, (n % 2) * 512
                    if n == 1:
                        nc.sync.dma_start(
                            out=wo_sb[:].rearrange("p a b -> p (a b)"),
                            in_=wob[:])
                    if n % 2 == 0 and pair + 1 < XP and n > 0:
                        load_x(pair + 1)
                    ps_q = [ps1.tile([128, 512], F32, tag=f"psq{m}", name=f"psq{m}")
                            for m in range(HPC)]
                    ps_k = [ps1.tile([128, 512], F32, tag=f"psk{m}", name=f"psk{m}")
                            for m in range(HPC)]
                    ps_v = [ps1.tile([128, 512], F32, tag=f"psv{i}", name=f"psv{i}")
                            for i in range(2)]
                    def qk_mms(g, kl):
                        kt = g * KTG + kl
                        xt = x_tiles[(pair, g)].rearrange(
                            "p (kt s) -> p kt s", kt=KTG)
                        xs = xt[:, kl, off:off + 512]
                        for m in range(HPC):
                            ms = slice(m * DH, (m + 1) * DH)
                            nc.tensor.matmul(ps_q[m][:], wq_g[g][:, kl, ms], xs,
                                             start=(kt == 0), stop=(kt == KT_D - 1),
                                             skip_group_check=True)
                        for m in range(HPC):
                            ms = slice(m * DH, (m + 1) * DH)
                            nc.tensor.matmul(ps_k[m][:], wk_g[g][:, kl, ms], xs,
                                             start=(kt == 0), stop=(kt == KT_D - 1),
                                             skip_group_check=True)

                    def v_mms(g, kl):
                        kt = g * KTG + kl
                        xt = x_tiles[(pair, g)].rearrange(
                            "p (kt s) -> p kt s", kt=KTG)
                        for sm in range(4):
                            pv = ps_v[sm // 2][:, (sm % 2) * 256:(sm % 2) * 256 + 256]
                            nc.tensor.matmul(pv,
                                             xt[:, kl, off + sm * 128:off + (sm + 1) * 128],
                                             wv_g[g][:, kl, :],
                                             start=(kt == 0 and sm % 2 == 0),
                                             stop=(kt == KT_D - 1),
                                             skip_group_check=True)

                    def rms_unit(ps_list, dst, w_col, m, sqeng, pe_bcast):
                        raw = ev.tile([128, 512], F32, tag="raw", bufs=3)
                        nc.vector.tensor_copy(raw[:], ps_list[m][:])
                        sq = ev.tile([128, 512], BF16, tag="sq", bufs=2)
                        if sqeng == "s":
                            nc.scalar.activation(sq[:], ps_list[m][:], AF.Square)
                        else:
                            nc.vector.tensor_mul(sq[:], raw[:], raw[:])
                        ps_ss = ps1.tile([1, 512], F32, tag="ss", name="ps_ss",
                                         bufs=2)
                        nc.tensor.matmul(ps_ss[:], ones_b[:], sq[:],
                                         start=True, stop=True,
                                         skip_group_check=True)
                        ms_row = ev.tile([1, 512], F32, tag="msr", bufs=2)
                        nc.scalar.activation(ms_row[:], ps_ss[:], AF.Identity,
                                             bias=eps_sb[:], scale=1.0 / 128.0)
                        rec = ev.tile([1, 512], F32, tag="rec", bufs=2)
                        nc.vector.reciprocal_approx_fast(out=rec[:], in_=ms_row[:])
                        if pe_bcast:
                            # broadcast on the PE, skipping gpsimd latency
                            rrms_b = ev.tile([1, 512], BF16, tag="rrms",
                                             bufs=2)
                            nc.scalar.activation(rrms_b[:], rec[:], AF.Sqrt)
                            rb_ps = ps1.tile([128, 512], F32, tag="ss",
                                             name="rb_ps", bufs=2)
                            nc.tensor.matmul(rb_ps[:], ones_r[:], rrms_b[:],
                                             start=True, stop=True,
                                             skip_group_check=True)
                            nc.vector.scalar_tensor_tensor(
                                dst[m][n][:], raw[:], w_col[:], rb_ps[:],
                                op0=MUL, op1=MUL)
                        else:
                            rrms = ev.tile([1, 512], F32R, tag="rrms", bufs=2)
                            nc.scalar.activation(rrms[:], rec[:], AF.Sqrt)
                            rb = ev.tile([128, 512], F32R, tag="rb", bufs=2)
                            nc.gpsimd.partition_broadcast(rb[:], rrms[:])
                            nc.vector.scalar_tensor_tensor(
                                dst[m][n][:], raw[:], w_col[:], rb[:],
                                op0=MUL, op1=MUL)

                    def v_evict():
                        for i in range(2):
                            nc.vector.tensor_copy(
                                v_sb[:, n * 4 + 2 * i:n * 4 + 2 * i + 2, :]
                                .rearrange("p a b -> p (a b)"),
                                ps_v[i][:])

                    if n < NCH - 1:
                        for g in range(KTG):
                            if n == 0 and g + 1 < KTG:
                                load_x_g(0, g + 1)
                                load_w_g(g + 1)
                            if n == 0 and g == KTG - 1:
                                load_x(1)
                            for kl in range(KTG):
                                qk_mms(g, kl)
                                v_mms(g, kl)
                        # q0 first in the mm order above hides the
                        # Square->ones-mm latency behind k/v matmuls
                        rms_unit(ps_q, qT, qw_sb, 0, "s", False)
                        rms_unit(ps_k, kT, kw_sb, 0, "v", False)
                        rms_unit(ps_q, qT, qw_sb, 1, "s", False)
                        rms_unit(ps_k, kT, kw_sb, 1, "v", False)
                        v_evict()
                    else:
                        # last chunk: sweep q/k first, start the h0 rms
                        # chains, then run the (independent) v matmuls under
                        # the rms drain so phase 2 starts right after the
                        # final matmul instead of after the whole drain
                        for g in range(KTG):
                            for kl in range(KTG):
                                qk_mms(g, kl)
                        rms_unit(ps_k, kT, kw_sb, 0, "s", True)
                        rms_unit(ps_q, qT, qw_sb, 0, "s", False)
                        for g in range(KTG):
                            for kl in range(KTG):
                                v_mms(g, kl)
                        rms_unit(ps_k, kT, kw_sb, 1, "s", False)
                        rms_unit(ps_q, qT, qw_sb, 1, "s", False)
                        v_evict()

            # ============ Phase 2+3: attention + out-projection ============
            # 1024-wide q blocks; S^T scores span two PSUM banks; exp and the
            # sum-exp accumulation amortize per-op overheads over 1024 cols.
            # PSUM budget (per-partition column space, 16KB total):
            #   pss 2x4KB + pso 4KB + y 2x2KB = 16KB.
            with (
                tc.tile_pool(name="ps2", bufs=1, space="PSUM") as ps2,
            ):
                NQB = S // 1024

                def outproj_half(qb, mo, u):
                    # one [128,512] half of the mo-th output row-block
                    mosl = slice(mo * 128, (mo + 1) * 128)
                    usl = slice(qb * 1024 + u * 512, qb * 1024 + (u + 1) * 512)
                    ps_y = ps2.tile([128, 512], F32, tag="y", name="ps_y",
                                    bufs=2)
                    for h2 in range(HPC):
                        nc.tensor.matmul(ps_y[:], wo_sb[:, h2, mosl],
                                         o_sb[h2][:, usl],
                                         start=(h2 == 0), stop=(h2 == HPC - 1),
                                         skip_group_check=True)
                    return ps_y

                def outproj_mo(qb, mo, tail=False):
                    y = stream.tile([128, 1024], BF16, tag="y_ev", bufs=3)
                    for u in range(2):
                        ps_y = outproj_half(qb, mo, u)
                        if tail and u == 0:
                            nc.scalar.copy(y[:, u * 512:(u + 1) * 512], ps_y[:])
                        else:
                            nc.vector.tensor_copy(y[:, u * 512:(u + 1) * 512],
                                                  ps_y[:])
                        if tail:
                            nc.sync.dma_start(
                                out=outT_t[mo][:, qb * 1024 + u * 512:
                                               qb * 1024 + (u + 1) * 512],
                                in_=y[:, u * 512:(u + 1) * 512])
                    if not tail:
                        nc.sync.dma_start(
                            out=outT_t[mo][:, qb * 1024:(qb + 1) * 1024],
                            in_=y[:])

                for qb in range(NQB):
                    qsl = slice(qb * 1024, (qb + 1) * 1024)
                    for h in range(HPC):
                        ps_o = ps2.tile([128, 1024], F32, tag="pso", bufs=1)
                        acc = ev.tile([128, 1024], BF16, tag="acc", bufs=2,
                                      name="acc")
                        pt_prev = None
                        def emit_pv(kt2, pt2):
                            for u in range(2):
                                nc.tensor.matmul(ps_o[:, u * 512:(u + 1) * 512],
                                                 v_sb[:, kt2, h * DH:(h + 1) * DH],
                                                 pt2[:, u * 512:(u + 1) * 512],
                                                 start=(kt2 == 0),
                                                 stop=(kt2 == KT_S - 1),
                                                 skip_group_check=True)

                        pv_pend = []
                        for kt in range(KT_S):
                            k_tile = kT[h][kt // 4][:, (kt % 4) * 128:
                                                    (kt % 4 + 1) * 128]
                            ps_s = ps2.tile([128, 1024], F32, tag="pss", bufs=2)
                            for u in range(2):
                                q_tile = qT[h][qb * 2 + u]
                                nc.tensor.matmul(ps_s[:, u * 512:(u + 1) * 512],
                                                 k_tile, q_tile[:],
                                                 start=True, stop=True,
                                                 skip_group_check=True)
                            pt = stream.tile([128, 1024], BF16, tag="pt", bufs=9)
                            nc.scalar.activation(pt[:], ps_s[:], AF.Exp, scale=SCALE)
                            if kt % 2 == 0:
                                pt_prev = pt
                            else:
                                pair = ev.tile([128, 1024], BF16, tag="pair",
                                               bufs=2, name="pair")
                                nc.vector.tensor_add(pair[:], pt_prev[:], pt[:])
                                if kt == 1:
                                    nc.vector.tensor_copy(acc[:], pair[:])
                                else:
                                    nc.vector.tensor_add(acc[:], acc[:], pair[:])
                            pv_pend.append((kt, pt))
                            if len(pv_pend) > 2:
                                emit_pv(*pv_pend.pop(0))
                            # interleave previous q-block's out-projection;
                            # deferred to kt>=3 so the qb-boundary softmax
                            # chain doesn't stall the score matmuls
                            if qb > 0:
                                if h == 0 and kt % 2 == 1 and kt >= 3:
                                    outproj_mo(qb - 1, (kt - 3) // 2)
                                elif h == 1 and kt == 1:
                                    outproj_mo(qb - 1, 15)
                        for kt2, pt2 in pv_pend:
                            emit_pv(kt2, pt2)
                        # evict the unnormalized PV sum immediately: frees the
                        # single pso accumulator for the next (qb,h) without
                        # waiting on the softmax-normalize chain
                        final = (qb == NQB - 1 and h == HPC - 1)
                        o_raw = ev.tile([128, 1024], F32, tag="oraw", bufs=1)
                        (nc.scalar.copy if final else nc.vector.tensor_copy)(
                            o_raw[:], ps_o[:])
                        # sum-exp bf16 ones-mm; reciprocal reads the PSUM
                        # row directly (no staging copy)
                        rec2 = ev.tile([1, 1024], F32, tag="rec2", bufs=1)
                        for u in range(2):
                            ps_se = ps2.tile([1, 512], F32, tag="y",
                                             name="ps_se", bufs=2)
                            nc.tensor.matmul(ps_se[:],
                                             ones_b[:],
                                             acc[:, u * 512:(u + 1) * 512],
                                             start=True, stop=True,
                                             skip_group_check=True)
                            nc.vector.reciprocal_approx_fast(
                                out=rec2[:, u * 512:(u + 1) * 512],
                                in_=ps_se[:])
                        if final:
                            # tail out-projection waits on this chain —
                            # broadcast via PE (bf16, 1/4 of the columns)
                            # instead of gpsimd
                            rec2b = ev.tile([1, 1024], BF16, tag="rrms",
                                            bufs=2)
                            nc.vector.tensor_copy(rec2b[:], rec2[:])
                            for u in range(2):
                                rb2_ps = ps2.tile([128, 512], F32, tag="y",
                                                  name="rb2_ps", bufs=2)
                                nc.tensor.matmul(
                                    rb2_ps[:], ones_r[:],
                                    rec2b[:, u * 512:(u + 1) * 512],
                                    start=True, stop=True,
                                    skip_group_check=True)
                                nc.vector.tensor_mul(
                                    o_sb[h][:, qb * 1024 + u * 512:
                                            qb * 1024 + (u + 1) * 512],
                                    o_raw[:, u * 512:(u + 1) * 512],
                                    rb2_ps[:])
                        else:
                            rb2 = ev.tile([128, 1024], F32, tag="rb2", bufs=1)
                            nc.gpsimd.partition_broadcast(rb2[:], rec2[:])
                            nc.vector.tensor_mul(o_sb[h][:, qsl], o_raw[:],
                                                 rb2[:])

                for mo in range(D // 128):
                    outproj_mo(NQB - 1, mo, tail=True)

    nc.compile()
    return nc


_NC_CACHE = None


def _get_nc():
    global _NC_CACHE
    if _NC_CACHE is None:
        _NC_CACHE = build()
    return _NC_CACHE


def _ensure_axon_hooks_stub():
    """bass_utils imports antenv.axon_hooks when tracing is requested via env;
    provide a no-op stub if the image lacks it so a stray BASS_TRACE cannot
    crash the run."""
    import types
    try:
        from antenv import axon_hooks  # noqa: F401
        return
    except Exception:
        pass
    try:
        import antenv
        m = types.ModuleType("antenv.axon_hooks")
        m.set_axon_ntff_profile_hook = lambda h: None
        m.get_axon_ntff_profile_hook = lambda: None
        sys.modules["antenv.axon_hooks"] = m
        antenv.axon_hooks = m
    except Exception:
        pass


def kernel(x, wq, wk, wv, wo, q_norm_w, k_norm_w):
    import ml_dtypes
    from concourse import bass_utils

    _ensure_axon_hooks_stub()

    x = np.asarray(x, dtype=np.float32)
    wq = np.asarray(wq, dtype=np.float32)
    wk = np.asarray(wk, dtype=np.float32)
    wv = np.asarray(wv, dtype=np.float32)
    wo = np.asarray(wo, dtype=np.float32)
    q_norm_w = np.asarray(q_norm_w, dtype=np.float32).reshape(DH, 1)
    k_norm_w = np.asarray(k_norm_w, dtype=np.float32).reshape(DH, 1)

    B = x.shape[0]
    # x^T packed [p, pair, kt, 1024] so DMA lines are 8KB
    xT = np.ascontiguousarray(x.reshape(S, D).T)
    xprep = np.ascontiguousarray(
        xT.reshape(KT_D, 128, XP, 1024).transpose(1, 2, 0, 3).reshape(
            128, XP * KT_D * 1024)).astype(ml_dtypes.bfloat16)

    def prep_w(wc):          # [2048, DHC] -> [128, kt*DHC] kt-packed
        return np.ascontiguousarray(
            wc.reshape(KT_D, 128, DHC).transpose(1, 0, 2).reshape(
                128, KT_D * DHC)).astype(ml_dtypes.bfloat16)

    in_maps = []
    for c in range(NC):
        hsl = slice(c * DHC, (c + 1) * DHC)
        woc = wo[:, hsl].T    # [DHC, D]
        wo_prep = np.ascontiguousarray(
            woc.reshape(HPC, 128, D).transpose(1, 0, 2).reshape(
                128, HPC * D)).astype(ml_dtypes.bfloat16)
        in_maps.append({
            "xTb": xprep,
            "wqb": prep_w(np.ascontiguousarray(wq[hsl, :].T)),
            "wkb": prep_w(np.ascontiguousarray(wk[hsl, :].T)),
            "wvb": prep_w(np.ascontiguousarray(wv[hsl, :].T)),
            "wob": wo_prep,
            "qw": q_norm_w,
            "kw": k_norm_w,
            "ones_c": np.ones((128, 1), dtype=ml_dtypes.bfloat16),
            "ones_r": np.ones((1, 128), dtype=ml_dtypes.bfloat16),
        })

    nc = _get_nc()
    res = bass_utils.run_bass_kernel_spmd(
        nc, in_maps, core_ids=list(range(NC)), trace=TRACE,
    )
    acc = np.zeros((D, S), dtype=np.float32)
    for c in range(NC):
        acc += res.results[c]["outT"].astype(np.float32)
    out = np.ascontiguousarray(acc.T).reshape(B, S, D)
    if TRACE:
        kernel.last_exec_time_ns = res.exec_time_ns
        kernel.last_results = res
    return out
